# revision 34
# baseline (speedup 1.0000x reference)
"""Additive (Bahdanau) attention on 8 TRN2 NeuronCores.

Reference math (per batch b):
    proj_enc[s,a] = sum_h enc[b,s,h] * W1[a,h]
    pd[a]        = sum_h dec[b,h]   * W2[a,h]
    energy       = tanh(proj_enc + pd)            # [S, A]
    score[s]     = sum_a energy[s,a] * v[a]
    w[s]         = exp(score[s]) * mask[b,s]      # no max-subtraction needed:
                                                  # |score| bounded by ||v||_1 (tanh in [-1,1])
    attn         = w / sum(w)
    ctx[h]       = sum_s w[s] * enc[b,s,h] / sum(w)

Sharding: data-parallel over batch B=64 -> 8 batches per core, weights
replicated, no collectives.

Per-core on-chip dataflow (B_loc=8, S=2048, H=1024, A=512), per (batch,
s-block of 512):
  - enc is shipped bf16 (host cast) and read TWICE from DRAM: once in natural
    [s, h] layout for the context matmul, once through the DMA xbar transpose
    engine straight into E^T layout [128p(h%128), 8(hc), 512(s)] for the
    projection matmul. Same total HBM bytes as one f32 read, and the PE never
    spends a cycle transposing.
  - proj^T computed in [a, s] layout: lhsT = W1T chunk [128h, 128a], rhs = E^T.
  - tanh+bias fused on ScalarE (bias = pd^T[a,b] per-partition), out bf16.
  - scores = v . energy via 4 accumulating matmuls (lhsT = v chunk [128,1]).
  - exp on ScalarE; mask multiply on VectorE writes the unnormalized w row.
  - w row -> w columns via K=1 matmuls (lhsT = w row slice [1,128], rhs = ones).
  - ctx accumulated across the whole batch in PSUM; the 4 s-subtile matmuls
    use 4 PE column-groups (tile_position) so their streams overlap; the 4
    partial rows are summed on the host together with the 1/sum(w) normalize.
"""

import os
import sys

sys.path.insert(0, "/opt/trn_rl_repo")

import numpy as np
import ml_dtypes

from concourse import bass, bacc, tile, mybir
from concourse.bass_utils import run_bass_kernel_spmd

F32 = mybir.dt.float32
BF16 = mybir.dt.bfloat16
AF = mybir.ActivationFunctionType
ALU = mybir.AluOpType

B, S, H, A = 64, 2048, 1024, 512
NCORES = 8
BL = B // NCORES          # batches per core

# "xbar": DMA crossbar transpose for E^T (off the PE). "pe": TensorE transpose.
TRANSPOSE_MODE = os.environ.get("ATTN_TRANSPOSE", "xbar")


def build_nc(bl=BL, s=S, transpose_mode=TRANSPOSE_MODE, debug_taps=False,
             reps=1, ctx_tile=True, sim_safe=False, bufs=3):
    # debug_taps: False | True (all) | set of tap names
    def tap_on(name):
        if debug_taps is True:
            return True
        if not debug_taps:
            return False
        return name in debug_taps
    SB = 512              # s-block size
    NBLK = s // SB        # s-blocks per batch
    NJ = SB // 128        # 128-row subtiles per s-block
    NHC = H // 128        # h chunks
    NAC = A // 128        # a chunks
    assert s % SB == 0

    nc = bacc.Bacc("TRN2", target_bir_lowering=False, debug=False,
                   num_devices=NCORES)

    enc = nc.dram_tensor("enc", [bl, s, H], BF16, kind="ExternalInput")
    w1t = nc.dram_tensor("w1t", [H, A], BF16, kind="ExternalInput")
    w2t = nc.dram_tensor("w2t", [H, A], BF16, kind="ExternalInput")
    dect = nc.dram_tensor("dect", [H, bl], BF16, kind="ExternalInput")
    vcol = nc.dram_tensor("vcol", [A, 1], BF16, kind="ExternalInput")
    maskf = nc.dram_tensor("maskf", [bl, s], F32, kind="ExternalInput")
    ones1 = nc.dram_tensor("ones1", [1, 1], F32, kind="ExternalInput")
    identf = nc.dram_tensor("identf", [128, 128], BF16, kind="ExternalInput")

    attn_o = nc.dram_tensor("attn_o", [bl, s], F32, kind="ExternalOutput")
    # 4 partial context rows per batch (one per col-group); host sums them.
    ctx_o = nc.dram_tensor("ctx_o", [bl, 4, H], F32, kind="ExternalOutput")
    if debug_taps:
        d_e = nc.dram_tensor("d_e", [128, 4, H], BF16, kind="ExternalOutput")
        d_et = nc.dram_tensor("d_et", [128, H // 128, 512], BF16, kind="ExternalOutput")
        d_en = nc.dram_tensor("d_en", [128, A // 128, 512], BF16, kind="ExternalOutput")
        d_exp = nc.dram_tensor("d_exp", [1, 512], F32, kind="ExternalOutput")
        d_w = nc.dram_tensor("d_w", [1, 512], F32, kind="ExternalOutput")
        d_wc = nc.dram_tensor("d_wc", [128, 4], F32, kind="ExternalOutput")
        d_ds = nc.dram_tensor("d_ds", [1, s // 512 + 2], F32, kind="ExternalOutput")
        d_pd = nc.dram_tensor("d_pd", [128, A // 128, bl], F32, kind="ExternalOutput")

    with tile.TileContext(nc) as tc:
        with (
            tc.tile_pool(name="consts", bufs=1) as consts,
            tc.tile_pool(name="pe_pool", bufs=bufs) as pe_pool,
            tc.tile_pool(name="pet", bufs=bufs) as pet,
            tc.tile_pool(name="pen", bufs=bufs) as pen,
            tc.tile_pool(name="pexp", bufs=2) as pexp,
            tc.tile_pool(name="pwcol", bufs=2) as pwcol,
            tc.tile_pool(name="pattn", bufs=2) as pattn,
            tc.tile_pool(name="pmask", bufs=2) as pmask,
            tc.tile_pool(name="pden", bufs=2) as pden,
            tc.tile_pool(name="pctxout", bufs=2) as pctxout,
            tc.tile_pool(name="pp_proj", bufs=2, space="PSUM") as pp_proj,
            tc.tile_pool(name="pp_sc", bufs=2, space="PSUM") as pp_sc,
            tc.tile_pool(name="pp_ctx", bufs=1, space="PSUM") as pp_ctx,
            tc.tile_pool(name="pp_wt", bufs=2, space="PSUM") as pp_wt,
        ):
            # ---- constants into SBUF ----
            w1t_sb = consts.tile([128, NHC, A], BF16, tag="w1t_sb")
            nc.sync.dma_start(w1t_sb[:], w1t.ap().rearrange("(c p) a -> p c a", p=128))
            w2t_sb = consts.tile([128, NHC, A], BF16, tag="w2t_sb")
            nc.sync.dma_start(w2t_sb[:], w2t.ap().rearrange("(c p) a -> p c a", p=128))
            dect_sb = consts.tile([128, NHC, bl], BF16, tag="dect_sb")
            nc.sync.dma_start(dect_sb[:], dect.ap().rearrange("(c p) b -> p c b", p=128))
            vcol_sb = consts.tile([128, NAC], BF16, tag="vcol_sb")
            nc.sync.dma_start(vcol_sb[:], vcol.ap().rearrange("(c p) o -> p (c o)", p=128))
            ones_sb = consts.tile([1, 1], F32, tag="ones_sb")
            nc.sync.dma_start(ones_sb[:], ones1.ap())
            ident_sb = consts.tile([128, 128], BF16, tag="ident_sb")
            nc.sync.dma_start(ident_sb[:], identf.ap())

            # ---- pd^T[a, b] = sum_h W2[a,h] dec[b,h] ----
            pdt_sb = consts.tile([128, NAC, bl], F32, tag="pdt_sb")
            for ac in range(NAC):
                pdp = pp_proj.tile([128, SB], F32, tag="pp", name="pdp")
                for hc in range(NHC):
                    nc.tensor.matmul(
                        pdp[:, :bl],
                        lhsT=w2t_sb[:, hc, ac * 128:(ac + 1) * 128],
                        rhs=dect_sb[:, hc, :],
                        start=(hc == 0), stop=(hc == NHC - 1),
                    )
                nc.vector.tensor_copy(pdt_sb[:, ac, :], pdp[:, :bl])

            # ---- main loop (optionally repeated on-device for timing) ----
            def main_body():
                for b in range(bl):
                    batch_body(b)

            def batch_body(b):
                mrow = pmask.tile([1, s], F32, name="mrow")
                nc.sync.dma_start(mrow[:], maskf[b:b + 1, :])
                attn_row = pattn.tile([1, s], F32, name="attn_row")
                nctxp = 128 if ctx_tile else 1
                ctx_ps = pp_ctx.tile([nctxp, 2, SB], F32, tag="ctx", name="ctx_ps")
                if ctx_tile and sim_safe:
                    # only partitions {0,32,64,96} are matmul targets; zero the
                    # rest so the simulator's uninit check passes (the DMA-out
                    # only reads the 4 written rows, so HW doesn't need this)
                    nc.vector.memset(ctx_ps[:], 0.0)

                for blk in range(NBLK):
                    s0 = blk * SB
                    # E block: s = s0 + j*128 + p
                    e = pe_pool.tile([128, NJ, H], BF16, tag="e", name="e")
                    nc.sync.dma_start(
                        out=e[:],
                        in_=enc[b, s0:s0 + SB, :].rearrange("(j p) h -> p j h", p=128),
                    )
                    # E^T block: h = hc*128 + p, read straight from DRAM
                    # through the xbar transpose engine.
                    et = pet.tile([128, NHC, SB], BF16, tag="et", name="et")
                    if transpose_mode == "xbar":
                        nc.sync.dma_start_transpose(
                            out=et[:],
                            in_=enc[b, s0:s0 + SB, :],
                        )
                    else:
                        for j in range(NJ):
                            for hcg in range(2):
                                tp = pp_wt.tile([128, 512], BF16, tag="tp",
                                                name="tp", bufs=1)
                                for k in range(4):
                                    hc = hcg * 4 + k
                                    nc.tensor.transpose(
                                        tp[:, k * 128:(k + 1) * 128],
                                        e[:, j, hc * 128:(hc + 1) * 128],
                                        ident_sb[:],
                                    )
                                dst = et[:, hcg * 4:(hcg + 1) * 4,
                                         j * 128:(j + 1) * 128]
                                if (j + hcg) % 2 == 0:
                                    nc.vector.tensor_copy(dst, tp[:].rearrange(
                                        "p (c q) -> p c q", c=4))
                                else:
                                    nc.scalar.copy(dst, tp[:].rearrange(
                                        "p (c q) -> p c q", c=4))

                    # proj^T chunks + fused bias/tanh
                    en = pen.tile([128, NAC, SB], BF16, tag="en", name="en")
                    for ac in range(NAC):
                        pp = pp_proj.tile([128, SB], F32, tag="pp", name="pp")
                        for hc in range(NHC):
                            nc.tensor.matmul(
                                pp[:],
                                lhsT=w1t_sb[:, hc, ac * 128:(ac + 1) * 128],
                                rhs=et[:, hc, :],
                                start=(hc == 0), stop=(hc == NHC - 1),
                            )
                        nc.scalar.activation(
                            en[:, ac, :], pp[:], AF.Tanh,
                            bias=pdt_sb[:, ac, b:b + 1],
                        )

                    # scores row [1, SB]
                    sc = pp_sc.tile([1, SB], F32, tag="sc", name="sc")
                    for ac in range(NAC):
                        nc.tensor.matmul(
                            sc[:],
                            lhsT=vcol_sb[:, ac:ac + 1],
                            rhs=en[:, ac, :],
                            start=(ac == 0), stop=(ac == NAC - 1),
                        )
                    exp_row = pexp.tile([1, SB], F32, name="exp_row")
                    nc.scalar.activation(exp_row[:], sc[:], AF.Exp)

                    # w = exp * mask into attn_row (unnormalized; the host
                    # divides by sum(w) — tensor_tensor_reduce hangs and
                    # tensor_scalar-with-AP races on this runtime build)
                    nc.vector.tensor_tensor(
                        out=attn_row[0:1, s0:s0 + SB],
                        in0=exp_row[:],
                        in1=mrow[0:1, s0:s0 + SB],
                        op=ALU.mult,
                    )

                    # w row -> w columns [128, NJ] via K=1 matmuls
                    wt = pp_wt.tile([128, 512], F32, tag="wt", name="wt",
                                    bufs=1 if transpose_mode == "pe" else None)
                    for j in range(NJ):
                        nc.tensor.matmul(
                            wt[:, j:j + 1],
                            lhsT=attn_row[0:1, s0 + j * 128: s0 + (j + 1) * 128],
                            rhs=ones_sb[:],
                            start=True, stop=True,
                        )
                    wcol = pwcol.tile([128, NJ], BF16, name="wcol")
                    nc.vector.tensor_copy(wcol[:], wt[:, :NJ])

                    if debug_taps and b == 0 and blk == 0:
                        if tap_on("e"): nc.sync.dma_start(d_e.ap(), e[:])
                        if tap_on("et"): nc.sync.dma_start(d_et.ap(), et[:])
                        if tap_on("en"): nc.sync.dma_start(d_en.ap(), en[:])
                        if tap_on("exp"): nc.sync.dma_start(d_exp.ap(), exp_row[:])
                        if tap_on("w"): nc.sync.dma_start(d_w.ap(), attn_row[0:1, 0:512])
                        if tap_on("wc"):
                            wcf = pwcol.tile([128, NJ], F32, tag="wcf", name="wcf")
                            nc.vector.tensor_copy(wcf[:], wt[:, :NJ])
                            nc.sync.dma_start(d_wc.ap(), wcf[:])
                        if tap_on("pd"): nc.sync.dma_start(d_pd.ap(), pdt_sb[:])

                    # ctx accumulation over the whole batch. The 4 j-subtile
                    # matmuls go to 4 PE column-groups (tile_position) so they
                    # stream concurrently; each col-group accumulates its own
                    # partial row (partition 32*j), summed on the host.
                    for j in range(NJ):
                        for hh in range(2):
                            if ctx_tile:
                                nc.tensor.matmul(
                                    ctx_ps[32 * j:32 * j + 1, hh, :],
                                    lhsT=wcol[:, j:j + 1],
                                    rhs=e[:, j, hh * SB:(hh + 1) * SB],
                                    start=(blk == 0),
                                    stop=(blk == NBLK - 1),
                                    tile_position=(0, 32 * j),
                                )
                            else:
                                nc.tensor.matmul(
                                    ctx_ps[0:1, hh, :],
                                    lhsT=wcol[:, j:j + 1],
                                    rhs=e[:, j, hh * SB:(hh + 1) * SB],
                                    start=(blk == 0 and j == 0),
                                    stop=(blk == NBLK - 1 and j == NJ - 1),
                                )

                # ---- batch epilogue: store unnormalized w and ctx ----
                nc.sync.dma_start(attn_o[b:b + 1, :], attn_row[:])
                ctxout = pctxout.tile([nctxp, 2, SB], F32, name="ctxout")
                nc.vector.tensor_copy(ctxout[:], ctx_ps[:])
                if ctx_tile:
                    nc.sync.dma_start(
                        ctx_o[b, :, :].rearrange("j (t q) -> j t q", t=2),
                        ctxout[0:97:32, :, :],
                    )
                else:
                    nc.sync.dma_start(
                        ctx_o[b, 0:1, :].rearrange("j (t q) -> j t q", t=2),
                        ctxout[:],
                    )

            if reps > 1:
                with tc.For_i(0, reps, 1):
                    main_body()
            else:
                main_body()

    nc.compile()
    return nc


_NC_CACHE = None


def _get_nc():
    global _NC_CACHE
    if _NC_CACHE is None:
        _NC_CACHE = build_nc()
    return _NC_CACHE


def make_core_inputs(dec_hidden, enc_outputs, enc_mask, W1, W2, v):
    """Per-core input dicts (host-side layout transforms + sharding only)."""
    dec_hidden = np.asarray(dec_hidden, dtype=np.float32)
    enc_outputs = np.asarray(enc_outputs, dtype=np.float32)
    enc_mask = np.asarray(enc_mask)
    W1 = np.asarray(W1, dtype=np.float32)
    W2 = np.asarray(W2, dtype=np.float32)
    v = np.asarray(v, dtype=np.float32)

    bf = ml_dtypes.bfloat16
    w1t = np.ascontiguousarray(W1.T).astype(bf)          # [H, A]
    w2t = np.ascontiguousarray(W2.T).astype(bf)          # [H, A]
    vcol = np.ascontiguousarray(v.reshape(A, 1)).astype(bf)
    ones1 = np.ones((1, 1), np.float32)
    identf = np.eye(128, dtype=np.float32).astype(bf)

    bl = enc_outputs.shape[0] // NCORES
    in_maps = []
    for c in range(NCORES):
        sl = slice(c * bl, (c + 1) * bl)
        in_maps.append({
            "enc": np.ascontiguousarray(enc_outputs[sl]).astype(bf),
            "w1t": w1t,
            "w2t": w2t,
            "dect": np.ascontiguousarray(dec_hidden[sl].T).astype(bf),  # [H, bl]
            "vcol": vcol,
            "maskf": np.ascontiguousarray(enc_mask[sl]).astype(np.float32),
            "ones1": ones1,
            "identf": identf,
        })
    return in_maps


def kernel(dec_hidden, enc_outputs, enc_mask, W1, W2, v, _want_results=False,
           **run_kwargs):
    nc = _get_nc()
    in_maps = make_core_inputs(dec_hidden, enc_outputs, enc_mask, W1, W2, v)
    res = run_bass_kernel_spmd(nc, in_maps, core_ids=list(range(NCORES)),
                               **run_kwargs)
    context = np.concatenate([r["ctx_o"] for r in res.results], axis=0).sum(axis=1)
    attn = np.concatenate([r["attn_o"] for r in res.results], axis=0)
    den = attn.sum(axis=1, keepdims=True)
    attn = attn / den
    context = context / den
    if _want_results:
        return (context, attn), res
    return (context, attn)


# revision 43
# speedup vs baseline: 1.1185x; 1.1185x over previous
"""Additive (Bahdanau) attention on 8 TRN2 NeuronCores.

Reference math (per batch b):
    proj_enc[s,a] = sum_h enc[b,s,h] * W1[a,h]
    pd[a]        = sum_h dec[b,h]   * W2[a,h]
    energy       = tanh(proj_enc + pd)            # [S, A]
    score[s]     = sum_a energy[s,a] * v[a]
    w[s]         = exp(score[s]) * mask[b,s]      # no max-subtraction needed:
                                                  # |score| bounded by ||v||_1 (tanh in [-1,1])
    attn         = w / sum(w)
    ctx[h]       = sum_s w[s] * enc[b,s,h] / sum(w)

Sharding: data-parallel over batch B=64 -> 8 batches per core, weights
replicated, no collectives.

Per-core on-chip dataflow (B_loc=8, S=2048, H=1024, A=512), per (batch,
s-block of 512):
  - enc is shipped bf16 (host cast) and read TWICE from DRAM: once in natural
    [s, h] layout for the context matmul, once through the DMA xbar transpose
    engine straight into E^T layout [128p(h%128), 8(hc), 512(s)] for the
    projection matmul. Same total HBM bytes as one f32 read, and the PE never
    spends a cycle transposing.
  - proj^T computed in [a, s] layout: lhsT = W1T chunk [128h, 128a], rhs = E^T.
  - tanh+bias fused on ScalarE (bias = pd^T[a,b] per-partition), out bf16.
  - scores = v . energy via 4 accumulating matmuls (lhsT = v chunk [128,1]).
  - exp on ScalarE; mask multiply on VectorE writes the unnormalized w row.
  - w row -> w columns via K=1 matmuls (lhsT = w row slice [1,128], rhs = ones).
  - ctx accumulated across the whole batch in PSUM; the 4 s-subtile matmuls
    use 4 PE column-groups (tile_position) so their streams overlap; the 4
    partial rows are summed on the host together with the 1/sum(w) normalize.
"""

import os
import sys

sys.path.insert(0, "/opt/trn_rl_repo")

import numpy as np
import ml_dtypes

from concourse import bass, bacc, tile, mybir
from concourse.bass_utils import run_bass_kernel_spmd

F32 = mybir.dt.float32
BF16 = mybir.dt.bfloat16
AF = mybir.ActivationFunctionType
ALU = mybir.AluOpType

B, S, H, A = 64, 2048, 1024, 512
NCORES = 8
BL = B // NCORES          # batches per core

# "xbar": DMA crossbar transpose for E^T (off the PE). "pe": TensorE transpose.
TRANSPOSE_MODE = os.environ.get("ATTN_TRANSPOSE", "xbar")


def build_nc(bl=BL, s=S, transpose_mode=TRANSPOSE_MODE, debug_taps=False,
             reps=1, ctx_tile=True, sim_safe=False, bufs=4):
    # debug_taps: False | True (all) | set of tap names
    def tap_on(name):
        if debug_taps is True:
            return True
        if not debug_taps:
            return False
        return name in debug_taps
    SB = 512              # s-block size
    NBLK = s // SB        # s-blocks per batch
    NJ = SB // 128        # 128-row subtiles per s-block
    NHC = H // 128        # h chunks
    NAC = A // 128        # a chunks
    assert s % SB == 0

    nc = bacc.Bacc("TRN2", target_bir_lowering=False, debug=False,
                   num_devices=NCORES)

    enc = nc.dram_tensor("enc", [bl, s, H], BF16, kind="ExternalInput")
    w1t = nc.dram_tensor("w1t", [H, A], BF16, kind="ExternalInput")
    w2t = nc.dram_tensor("w2t", [H, A], BF16, kind="ExternalInput")
    dect = nc.dram_tensor("dect", [H, bl], BF16, kind="ExternalInput")
    vcol = nc.dram_tensor("vcol", [A, 1], BF16, kind="ExternalInput")
    maskf = nc.dram_tensor("maskf", [bl, s], F32, kind="ExternalInput")
    ones1 = nc.dram_tensor("ones1", [1, 1], F32, kind="ExternalInput")
    identf = nc.dram_tensor("identf", [128, 128], BF16, kind="ExternalInput")

    attn_o = nc.dram_tensor("attn_o", [bl, s], F32, kind="ExternalOutput")
    # 4 partial context rows per batch (one per col-group); host sums them.
    ctx_o = nc.dram_tensor("ctx_o", [bl, 4, H], F32, kind="ExternalOutput")
    if debug_taps:
        d_e = nc.dram_tensor("d_e", [128, 4, H], BF16, kind="ExternalOutput")
        d_et = nc.dram_tensor("d_et", [128, H // 128, 512], BF16, kind="ExternalOutput")
        d_en = nc.dram_tensor("d_en", [128, A // 128, 512], BF16, kind="ExternalOutput")
        d_exp = nc.dram_tensor("d_exp", [1, 512], F32, kind="ExternalOutput")
        d_w = nc.dram_tensor("d_w", [1, 512], F32, kind="ExternalOutput")
        d_wc = nc.dram_tensor("d_wc", [128, 4], F32, kind="ExternalOutput")
        d_ds = nc.dram_tensor("d_ds", [1, s // 512 + 2], F32, kind="ExternalOutput")
        d_pd = nc.dram_tensor("d_pd", [128, A // 128, bl], F32, kind="ExternalOutput")

    with tile.TileContext(nc) as tc:
        with (
            tc.tile_pool(name="consts", bufs=1) as consts,
            tc.tile_pool(name="pe_pool", bufs=bufs) as pe_pool,
            tc.tile_pool(name="pet", bufs=bufs) as pet,
            tc.tile_pool(name="pen", bufs=bufs) as pen,
            tc.tile_pool(name="pexp", bufs=2) as pexp,
            tc.tile_pool(name="pwcol", bufs=2) as pwcol,
            tc.tile_pool(name="pattn", bufs=2) as pattn,
            tc.tile_pool(name="pmask", bufs=2) as pmask,
            tc.tile_pool(name="pden", bufs=2) as pden,
            tc.tile_pool(name="pctxout", bufs=2) as pctxout,
            tc.tile_pool(name="pp_proj", bufs=2, space="PSUM") as pp_proj,
            tc.tile_pool(name="pp_sc", bufs=2, space="PSUM") as pp_sc,
            tc.tile_pool(name="pp_ctx", bufs=1, space="PSUM") as pp_ctx,
            tc.tile_pool(name="pp_wt", bufs=2, space="PSUM") as pp_wt,
        ):
            # ---- constants into SBUF ----
            w1t_sb = consts.tile([128, NHC, A], BF16, tag="w1t_sb")
            nc.sync.dma_start(w1t_sb[:], w1t.ap().rearrange("(c p) a -> p c a", p=128))
            w2t_sb = consts.tile([128, NHC, A], BF16, tag="w2t_sb")
            nc.sync.dma_start(w2t_sb[:], w2t.ap().rearrange("(c p) a -> p c a", p=128))
            dect_sb = consts.tile([128, NHC, bl], BF16, tag="dect_sb")
            nc.sync.dma_start(dect_sb[:], dect.ap().rearrange("(c p) b -> p c b", p=128))
            vcol_sb = consts.tile([128, NAC], BF16, tag="vcol_sb")
            nc.sync.dma_start(vcol_sb[:], vcol.ap().rearrange("(c p) o -> p (c o)", p=128))
            ones_sb = consts.tile([1, 1], F32, tag="ones_sb")
            nc.sync.dma_start(ones_sb[:], ones1.ap())
            ident_sb = consts.tile([128, 128], BF16, tag="ident_sb")
            nc.sync.dma_start(ident_sb[:], identf.ap())

            # ---- pd^T[a, b] = sum_h W2[a,h] dec[b,h] ----
            pdt_sb = consts.tile([128, NAC, bl], F32, tag="pdt_sb")
            for ac in range(NAC):
                pdp = pp_proj.tile([128, SB], F32, tag="pp", name="pdp")
                for hc in range(NHC):
                    nc.tensor.matmul(
                        pdp[:, :bl],
                        lhsT=w2t_sb[:, hc, ac * 128:(ac + 1) * 128],
                        rhs=dect_sb[:, hc, :],
                        start=(hc == 0), stop=(hc == NHC - 1),
                    )
                nc.vector.tensor_copy(pdt_sb[:, ac, :], pdp[:, :bl])

            # ---- main loop (optionally repeated on-device for timing) ----
            # Deferred-work queue (closures). Block k's w-column/ctx matmuls
            # depend on ScalarE exp + VectorE mask of block k; emitting them
            # immediately would stall the in-order PE stream. They are queued
            # and flushed after the NEXT block's projection matmuls, whose
            # issue covers the latency. The queue crosses batch boundaries:
            # the per-batch epilogue is queued behind the batch's last
            # wt/ctx closure so the PE never waits at a batch tail either.
            pending = []

            def flush_pending():
                while pending:
                    pending.pop(0)()

            def main_body():
                for b in range(bl):
                    batch_body(b)
                flush_pending()

            def batch_body(b):
                mrow = pmask.tile([1, s], F32, name="mrow")
                nc.sync.dma_start(mrow[:], maskf[b:b + 1, :])
                attn_row = pattn.tile([1, s], F32, name="attn_row")
                nctxp = 128 if ctx_tile else 1
                ctx_ps = pp_ctx.tile([nctxp, 2, SB], F32, tag="ctx", name="ctx_ps")
                if ctx_tile and sim_safe:
                    # only partitions {0,32,64,96} are matmul targets; zero the
                    # rest so the simulator's uninit check passes (the DMA-out
                    # only reads the 4 written rows, so HW doesn't need this).
                    # Queued so it lands after the previous batch's epilogue
                    # read of the same PSUM slot.
                    pending.append(lambda: nc.vector.memset(ctx_ps[:], 0.0))

                def emit_wt_ctx(pblk, pe_tile):
                    ps0 = pblk * SB
                    wt = pp_wt.tile([128, 512], F32, tag="wt", name="wt",
                                    bufs=1 if transpose_mode == "pe" else None)
                    for j in range(NJ):
                        nc.tensor.matmul(
                            wt[:, j:j + 1],
                            lhsT=attn_row[0:1, ps0 + j * 128: ps0 + (j + 1) * 128],
                            rhs=ones_sb[:],
                            start=True, stop=True,
                        )
                    wcol = pwcol.tile([128, NJ], BF16, name="wcol")
                    nc.vector.tensor_copy(wcol[:], wt[:, :NJ])
                    for j in range(NJ):
                        for hh in range(2):
                            if ctx_tile:
                                nc.tensor.matmul(
                                    ctx_ps[32 * j:32 * j + 1, hh, :],
                                    lhsT=wcol[:, j:j + 1],
                                    rhs=pe_tile[:, j, hh * SB:(hh + 1) * SB],
                                    start=(pblk == 0),
                                    stop=(pblk == NBLK - 1),
                                    tile_position=(0, 32 * j),
                                )
                            else:
                                nc.tensor.matmul(
                                    ctx_ps[0:1, hh, :],
                                    lhsT=wcol[:, j:j + 1],
                                    rhs=pe_tile[:, j, hh * SB:(hh + 1) * SB],
                                    start=(pblk == 0 and j == 0),
                                    stop=(pblk == NBLK - 1 and j == NJ - 1),
                                )

                for blk in range(NBLK):
                    s0 = blk * SB
                    # E block: s = s0 + j*128 + p
                    e = pe_pool.tile([128, NJ, H], BF16, tag="e", name="e")
                    nc.sync.dma_start(
                        out=e[:],
                        in_=enc[b, s0:s0 + SB, :].rearrange("(j p) h -> p j h", p=128),
                    )
                    # E^T block: h = hc*128 + p, read straight from DRAM
                    # through the xbar transpose engine.
                    et = pet.tile([128, NHC, SB], BF16, tag="et", name="et")
                    if transpose_mode == "xbar":
                        nc.sync.dma_start_transpose(
                            out=et[:],
                            in_=enc[b, s0:s0 + SB, :],
                        )
                    else:
                        for j in range(NJ):
                            for hcg in range(2):
                                tp = pp_wt.tile([128, 512], BF16, tag="tp",
                                                name="tp", bufs=1)
                                for k in range(4):
                                    hc = hcg * 4 + k
                                    nc.tensor.transpose(
                                        tp[:, k * 128:(k + 1) * 128],
                                        e[:, j, hc * 128:(hc + 1) * 128],
                                        ident_sb[:],
                                    )
                                dst = et[:, hcg * 4:(hcg + 1) * 4,
                                         j * 128:(j + 1) * 128]
                                if (j + hcg) % 2 == 0:
                                    nc.vector.tensor_copy(dst, tp[:].rearrange(
                                        "p (c q) -> p c q", c=4))
                                else:
                                    nc.scalar.copy(dst, tp[:].rearrange(
                                        "p (c q) -> p c q", c=4))

                    # proj^T chunks + fused bias/tanh
                    en = pen.tile([128, NAC, SB], BF16, tag="en", name="en")
                    for ac in range(NAC):
                        pp = pp_proj.tile([128, SB], F32, tag="pp", name="pp")
                        for hc in range(NHC):
                            nc.tensor.matmul(
                                pp[:],
                                lhsT=w1t_sb[:, hc, ac * 128:(ac + 1) * 128],
                                rhs=et[:, hc, :],
                                start=(hc == 0), stop=(hc == NHC - 1),
                            )
                        nc.scalar.activation(
                            en[:, ac, :], pp[:], AF.Tanh,
                            bias=pdt_sb[:, ac, b:b + 1],
                        )

                    # previous block's deferred PE work goes here: its exp/mask
                    # dependencies resolved a block ago, and it covers the
                    # tanh latency this block's scores matmuls wait on
                    flush_pending()

                    # scores row [1, SB]
                    sc = pp_sc.tile([1, SB], F32, tag="sc", name="sc")
                    for ac in range(NAC):
                        nc.tensor.matmul(
                            sc[:],
                            lhsT=vcol_sb[:, ac:ac + 1],
                            rhs=en[:, ac, :],
                            start=(ac == 0), stop=(ac == NAC - 1),
                        )
                    exp_row = pexp.tile([1, SB], F32, name="exp_row")
                    nc.scalar.activation(exp_row[:], sc[:], AF.Exp)

                    # w = exp * mask into attn_row (unnormalized; the host
                    # divides by sum(w) — tensor_tensor_reduce hangs and
                    # tensor_scalar-with-AP races on this runtime build)
                    nc.vector.tensor_tensor(
                        out=attn_row[0:1, s0:s0 + SB],
                        in0=exp_row[:],
                        in1=mrow[0:1, s0:s0 + SB],
                        op=ALU.mult,
                    )

                    if debug_taps and b == 0 and blk == 0:
                        if tap_on("e"): nc.sync.dma_start(d_e.ap(), e[:])
                        if tap_on("et"): nc.sync.dma_start(d_et.ap(), et[:])
                        if tap_on("en"): nc.sync.dma_start(d_en.ap(), en[:])
                        if tap_on("exp"): nc.sync.dma_start(d_exp.ap(), exp_row[:])
                        if tap_on("w"): nc.sync.dma_start(d_w.ap(), attn_row[0:1, 0:512])
                        if tap_on("pd"): nc.sync.dma_start(d_pd.ap(), pdt_sb[:])

                    pending.append(
                        lambda pblk=blk, pe_tile=e: emit_wt_ctx(pblk, pe_tile))

                def epilogue():
                    # store unnormalized w and the 4 partial ctx rows
                    nc.sync.dma_start(attn_o[b:b + 1, :], attn_row[:])
                    ctxout = pctxout.tile([nctxp, 2, SB], F32, name="ctxout")
                    nc.vector.tensor_copy(ctxout[:], ctx_ps[:])
                    if ctx_tile:
                        nc.sync.dma_start(
                            ctx_o[b, :, :].rearrange("j (t q) -> j t q", t=2),
                            ctxout[0:97:32, :, :],
                        )
                    else:
                        nc.sync.dma_start(
                            ctx_o[b, 0:1, :].rearrange("j (t q) -> j t q", t=2),
                            ctxout[:],
                        )

                # queued behind this batch's last wt/ctx closure; flushed
                # during the next batch (or at the end of main_body)
                pending.append(epilogue)

            if reps > 1:
                with tc.For_i(0, reps, 1):
                    main_body()
            else:
                main_body()

    nc.compile()
    return nc


_NC_CACHE = None


def _get_nc():
    global _NC_CACHE
    if _NC_CACHE is None:
        _NC_CACHE = build_nc()
    return _NC_CACHE


def make_core_inputs(dec_hidden, enc_outputs, enc_mask, W1, W2, v):
    """Per-core input dicts (host-side layout transforms + sharding only)."""
    dec_hidden = np.asarray(dec_hidden, dtype=np.float32)
    enc_outputs = np.asarray(enc_outputs, dtype=np.float32)
    enc_mask = np.asarray(enc_mask)
    W1 = np.asarray(W1, dtype=np.float32)
    W2 = np.asarray(W2, dtype=np.float32)
    v = np.asarray(v, dtype=np.float32)

    bf = ml_dtypes.bfloat16
    w1t = np.ascontiguousarray(W1.T).astype(bf)          # [H, A]
    w2t = np.ascontiguousarray(W2.T).astype(bf)          # [H, A]
    vcol = np.ascontiguousarray(v.reshape(A, 1)).astype(bf)
    ones1 = np.ones((1, 1), np.float32)
    identf = np.eye(128, dtype=np.float32).astype(bf)

    bl = enc_outputs.shape[0] // NCORES
    in_maps = []
    for c in range(NCORES):
        sl = slice(c * bl, (c + 1) * bl)
        in_maps.append({
            "enc": np.ascontiguousarray(enc_outputs[sl]).astype(bf),
            "w1t": w1t,
            "w2t": w2t,
            "dect": np.ascontiguousarray(dec_hidden[sl].T).astype(bf),  # [H, bl]
            "vcol": vcol,
            "maskf": np.ascontiguousarray(enc_mask[sl]).astype(np.float32),
            "ones1": ones1,
            "identf": identf,
        })
    return in_maps


def kernel(dec_hidden, enc_outputs, enc_mask, W1, W2, v, _want_results=False,
           **run_kwargs):
    nc = _get_nc()
    in_maps = make_core_inputs(dec_hidden, enc_outputs, enc_mask, W1, W2, v)
    res = run_bass_kernel_spmd(nc, in_maps, core_ids=list(range(NCORES)),
                               **run_kwargs)
    context = np.concatenate([r["ctx_o"] for r in res.results], axis=0).sum(axis=1)
    attn = np.concatenate([r["attn_o"] for r in res.results], axis=0)
    den = attn.sum(axis=1, keepdims=True)
    attn = attn / den
    context = context / den
    if _want_results:
        return (context, attn), res
    return (context, attn)
